# revision 1
# baseline (speedup 1.0000x reference)
"""MultiEdgeGraphBlock kernel for 8 Trainium2 NeuronCores.

Design (per core; sharding = (batch b, node-half) -> 8 cores):
  - Neighbor gather with SWDGE dma_gather (non-transpose, NODE-major output)
    from an HBM bf16 table (node-major rows, +1 zero row for masked-out
    edges). 2048-index calls round-robined over 4 SWDGE queues reach the
    SBUF fabric rate (~300-450 GB/s); transpose-mode gathers corrupt when
    run on concurrent queues, node-major mode is safe.
  - Per gather call, G[p, d, :] = h[idx(node p, neighbor d), :]. The
    16-neighbor reduction runs on the PE as 16 identity-weight matmuls
    accumulating in fp32 PSUM (layout-agnostic passthrough accumulate).
  - Per-edge-type mean: reciprocal degree lives node-major (one value per
    partition), applied as a per-partition ACT scale during PSUM eviction.
  - The reduced means (16x smaller than the gathered volume) are transposed
    to feature-major with PE transpose, then aggregated = sum_i W_i^T @
    meanT_i accumulates in PSUM.
  - LayerNorm in feature-major form: stats via ones-vector matmuls (bf16),
    mu/rstd rows broadcast across partitions with a step-0 DMA from DRAM
    scratch; MLP as feature-major bf16 matmuls; residual add in f32.
"""

import sys

sys.path.insert(0, "/opt/trn_rl_repo")

import numpy as np
import ml_dtypes
from contextlib import ExitStack

import concourse.bass as bass
import concourse.mybir as mybir
import concourse.tile as tile
from concourse import bacc
from concourse.bass_utils import run_bass_kernel_spmd

BF16 = ml_dtypes.bfloat16
F32 = mybir.dt.float32
BF = mybir.dt.bfloat16
I16 = mybir.dt.int16
AO = mybir.AluOpType
AF = mybir.ActivationFunctionType

B, N, F, E, DEG, H = 4, 10000, 256, 5, 16, 256
NLOC = N // 2          # nodes per core (node-half sharding)
BLK = 512              # node block for LN/MLP (matmul free dim)
NPAD = 5120            # NLOC padded to multiple of BLK
NBLK = NPAD // BLK     # 10
NB1 = NPAD // 128      # 40 gather-blocks of 128 nodes
ZROW = N               # zero-row index in gather table
GIDX = DEG * 128       # 2048 indices per gather call
NQUEUES = 4
LN_EPS = 1e-6

_PROGRAM = {}
VARIANT = "full"  # full | gather | nogather


def _build_program(repeat=1):
    nc = bacc.Bacc(
        "TRN2",
        target_bir_lowering=False,
        debug=False,
        dynamic_dma_scratch_size=32768,
        num_swdge_queues=NQUEUES,
    )

    tbl = nc.dram_tensor("tbl", [N + 1, F], BF, kind="ExternalInput")
    hT = nc.dram_tensor("hT", [F, NPAD], F32, kind="ExternalInput")
    idxw = nc.dram_tensor("idxw", [E, NB1, 128, 128], I16, kind="ExternalInput")
    masknm = nc.dram_tensor("masknm", [128, E, NB1, DEG], F32, kind="ExternalInput")
    w_pe = nc.dram_tensor("w_pe", [128, E * 4, 128], BF, kind="ExternalInput")
    w1 = nc.dram_tensor("w1", [128, 8, 128], BF, kind="ExternalInput")
    w2 = nc.dram_tensor("w2", [128, 4, 128], BF, kind="ExternalInput")
    ident_d = nc.dram_tensor("ident", [128, 128], BF, kind="ExternalInput")
    ones1_d = nc.dram_tensor("ones1", [128, 1], BF, kind="ExternalInput")
    b1_d = nc.dram_tensor("b1pc", [128, 2], F32, kind="ExternalInput")
    b2_d = nc.dram_tensor("b2pc", [128, 2], F32, kind="ExternalInput")
    lns_d = nc.dram_tensor("lnspc", [128, 4], F32, kind="ExternalInput")
    lnb_d = nc.dram_tensor("lnbpc", [128, 4], F32, kind="ExternalInput")
    bedg_d = nc.dram_tensor("bedgpc", [128, 2, E], F32, kind="ExternalInput")

    outT = nc.dram_tensor("outT", [F, NPAD], F32, kind="ExternalOutput")

    with tile.TileContext(nc) as tc, ExitStack() as ctx:
        cpool = ctx.enter_context(tc.tile_pool(name="const", bufs=1))
        spsum = ctx.enter_context(tc.tile_pool(name="spsum", bufs=2, space="PSUM"))
        mtpsum = ctx.enter_context(tc.tile_pool(name="mtpsum", bufs=2, space="PSUM"))
        apsum = ctx.enter_context(tc.tile_pool(name="apsum", bufs=1, space="PSUM"))
        mlpsum = ctx.enter_context(tc.tile_pool(name="mlpsum", bufs=1, space="PSUM"))
        dpool = ctx.enter_context(tc.tile_pool(name="dram", bufs=2, space="DRAM"))
        gpool = ctx.enter_context(tc.tile_pool(name="g", bufs=4))
        ipool = ctx.enter_context(tc.tile_pool(name="idx", bufs=4))
        xpool = ctx.enter_context(tc.tile_pool(name="x", bufs=2))
        wpool = ctx.enter_context(tc.tile_pool(name="work", bufs=2))

        # ---------------- constants ----------------
        W_sb = cpool.tile([128, E * 4, 128], BF)
        nc.sync.dma_start(W_sb[:], w_pe[:])
        W1_sb = cpool.tile([128, 8, 128], BF)
        nc.sync.dma_start(W1_sb[:], w1[:])
        W2_sb = cpool.tile([128, 4, 128], BF)
        nc.sync.dma_start(W2_sb[:], w2[:])
        id_sb = cpool.tile([128, 128], BF)
        nc.sync.dma_start(id_sb[:], ident_d[:])
        on_sb = cpool.tile([128, 1], BF)
        nc.sync.dma_start(on_sb[:], ones1_d[:])
        b1_sb = cpool.tile([128, 2], F32)
        nc.sync.dma_start(b1_sb[:], b1_d[:])
        b2_sb = cpool.tile([128, 2], F32)
        nc.sync.dma_start(b2_sb[:], b2_d[:])
        lns_sb = cpool.tile([128, 4], F32)
        nc.sync.dma_start(lns_sb[:], lns_d[:])
        lnb_sb = cpool.tile([128, 4], F32)
        nc.sync.dma_start(lnb_sb[:], lnb_d[:])
        bedg_sb = cpool.tile([128, 2, E], F32)
        nc.sync.dma_start(bedg_sb[:], bedg_d[:])
        bsum_sb = cpool.tile([128, 2], F32)
        nc.vector.tensor_reduce(
            bsum_sb[:], bedg_sb[:], axis=mybir.AxisListType.X, op=AO.add
        )

        # ---------------- reciprocal degree (node-major) ----------------
        mask_sb = cpool.tile([128, E, NB1, DEG], F32)
        nc.sync.dma_start(mask_sb[:], masknm[:])
        dn_sb = cpool.tile([128, E, NB1], F32)
        for i in range(E):
            nc.vector.tensor_reduce(
                dn_sb[:, i, :], mask_sb[:, i], axis=mybir.AxisListType.X, op=AO.add
            )
        nc.vector.tensor_scalar_max(dn_sb[:], dn_sb[:], 1.0)
        recip_sb = cpool.tile([128, E, NB1], F32)
        nc.vector.reciprocal(recip_sb[:], dn_sb[:])

        qc = 0  # SWDGE queue round-robin counter

        # ---------------- main loop over node blocks ----------------
        for rep in range(repeat):
            for blk in range(NBLK):
                ns = bass.ts(blk, BLK)
                x = xpool.tile([128, 4, BLK], F32)
                nc.sync.dma_start(x[:, 0, :], hT[0:128, ns])
                nc.sync.dma_start(x[:, 1, :], hT[128:256, ns])

                for sub in range(4):
                    nb = blk * 4 + sub  # 128-node gather block
                    agg = apsum.tile([128, 2, 512], F32, tag="agg")  # m-slices in separate banks
                    for i in range(E):
                        idx_t = ipool.tile([128, 128], I16)
                        nc.sync.dma_start(idx_t[:], idxw[i, nb])
                        G = gpool.tile([128, DEG, F], BF, tag="G")
                        ga = G[:]
                        gap = bass.AP(
                            ga.tensor, ga.offset, [ga.ap[0], [F, DEG], [1, F]]
                        )
                        if VARIANT == "nogather":
                            pass
                        else:
                            nc.gpsimd.dma_gather(
                                out_ap=gap,
                                in_ap=tbl.ap(),
                                idxs_ap=idx_t[:],
                                num_idxs=GIDX,
                                num_idxs_reg=GIDX,
                                elem_size=F,
                                single_packet=False,
                                queue_num=qc % NQUEUES,
                            )
                        qc += 1
                        S = spsum.tile([128, F], F32, tag="S")
                        ND = 1 if VARIANT == "gather" else DEG
                        for d in range(ND):
                            nc.tensor.matmul(
                                S[:],
                                id_sb[:],
                                G[:, d, :],
                                start=(d == 0),
                                stop=(d == ND - 1),
                            )
                        if VARIANT == "gather":
                            continue
                        # mean (node-major): per-partition reciprocal scale
                        mean = wpool.tile([128, F], BF, tag="mean")
                        nc.scalar.activation(
                            mean[:], S[:], AF.Copy,
                            scale=recip_sb[:, i, nb : nb + 1],
                        )
                        # transpose reduced means to feature-major
                        mT = mtpsum.tile([128, 2, 128], BF, tag="mT")
                        for c in range(2):
                            nc.tensor.transpose(
                                mT[:, c, :], mean[:, c * 128 : (c + 1) * 128],
                                id_sb[:],
                            )
                        mT_sb = wpool.tile([128, 2, 128], BF, tag="mTsb")
                        nc.scalar.copy(mT_sb[:], mT[:])
                        for m in range(2):
                            for c in range(2):
                                nc.tensor.matmul(
                                    agg[:, m, 0:128],
                                    W_sb[:, (i * 2 + c) * 2 + m, :],
                                    mT_sb[:, c, :],
                                    start=(i == 0 and c == 0),
                                    stop=(i == E - 1 and c == 1),
                                )
                    # aggregated -> x bottom half (+ sum of edge biases)
                    for m in range(2 if VARIANT != "gather" else 0):
                        nc.scalar.activation(
                            x[:, 2 + m, sub * 128 : (sub + 1) * 128],
                            agg[:, m, 0:128],
                            AF.Identity,
                            bias=bsum_sb[:, m : m + 1],
                            scale=1.0,
                        )

                # ---------------- layer norm (features on partitions) ------
                if VARIANT == "gather":
                    continue
                st = mlpsum.tile([128, 2, BLK], F32, tag="mlp")
                xbs = []
                for c in range(4):
                    xb = wpool.tile([128, BLK], BF, tag=f"xb{c}")
                    nc.vector.tensor_copy(xb[:], x[:, c, :])
                    xbs.append(xb)
                    nc.tensor.matmul(
                        st[0:1, 0, :], on_sb[:], xb[:],
                        start=(c == 0), stop=(c == 3),
                    )
                for c in range(4):
                    xsq = wpool.tile([128, BLK], BF, tag="xsq")
                    nc.scalar.square(xsq[:], xbs[c][:])
                    nc.tensor.matmul(
                        st[0:1, 1, :], on_sb[:], xsq[:],
                        start=(c == 0), stop=(c == 3),
                    )
                mrow = wpool.tile([1, 2, BLK], F32, tag="mrow")
                nc.vector.tensor_scalar_mul(mrow[0:1, 0, :], st[0:1, 0, :], 1.0 / 512.0)
                mu2 = wpool.tile([1, BLK], F32, tag="mu2")
                nc.vector.tensor_mul(mu2[0:1, :], mrow[0:1, 0, :], mrow[0:1, 0, :])
                nc.vector.tensor_scalar_sub(mu2[0:1, :], mu2[0:1, :], LN_EPS)
                var = wpool.tile([1, BLK], F32, tag="var")
                nc.vector.scalar_tensor_tensor(
                    var[0:1, :], st[0:1, 1, :], 1.0 / 512.0, mu2[0:1, :],
                    op0=AO.mult, op1=AO.subtract,
                )
                sd = wpool.tile([1, BLK], F32, tag="sd")
                nc.scalar.activation(sd[0:1, :], var[0:1, :], AF.Sqrt, bias=0.0)
                nc.vector.reciprocal(mrow[0:1, 1, :], sd[0:1, :])
                mrow_d = dpool.tile([1, 2, BLK], F32, tag="mrowd")
                nc.sync.dma_start(mrow_d[:], mrow[:])
                mq = wpool.tile([128, 2, BLK], F32, tag="mq")
                msrc = bass.AP(
                    mrow_d.tensor, mrow_d[:].offset, [[0, 128], [BLK, 2], [1, BLK]]
                )
                nc.sync.dma_start(mq[:], msrc)

                xln = wpool.tile([128, 4, BLK], BF, tag="xln")
                for c in range(4):
                    tt = wpool.tile([128, BLK], F32, tag="tt")
                    nc.vector.scalar_tensor_tensor(
                        tt[:], x[:, c, :], 0.0, mq[:, 0, :],
                        op0=AO.add, op1=AO.subtract,
                    )
                    nc.vector.tensor_mul(tt[:], tt[:], mq[:, 1, :])
                    nc.scalar.activation(
                        xln[:, c, :], tt[:], AF.Identity,
                        bias=lnb_sb[:, c : c + 1], scale=lns_sb[:, c : c + 1],
                    )

                # ---------------- MLP ----------------
                y1 = mlpsum.tile([128, 2, BLK], F32, tag="mlp")
                for m in range(2):
                    for k in range(4):
                        nc.tensor.matmul(
                            y1[:, m, :], W1_sb[:, k * 2 + m, :], xln[:, k, :],
                            start=(k == 0), stop=(k == 3),
                        )
                y1b = wpool.tile([128, 2, BLK], BF, tag="y1b")
                for m in range(2):
                    nc.scalar.activation(
                        y1b[:, m, :], y1[:, m, :], AF.Relu,
                        bias=b1_sb[:, m : m + 1], scale=1.0,
                    )
                y2 = mlpsum.tile([128, 2, BLK], F32, tag="mlp")
                for m in range(2):
                    for k in range(2):
                        nc.tensor.matmul(
                            y2[:, m, :], W2_sb[:, k * 2 + m, :], y1b[:, k, :],
                            start=(k == 0), stop=(k == 1),
                        )
                ot = wpool.tile([128, 2, BLK], F32, tag="ot")
                for m in range(2):
                    nc.vector.scalar_tensor_tensor(
                        ot[:, m, :], y2[:, m, :], b2_sb[:, m : m + 1], x[:, m, :],
                        op0=AO.add, op1=AO.add,
                    )
                for m in range(2):
                    nc.sync.dma_start(outT[m * 128 : (m + 1) * 128, ns], ot[:, m, :])

    nc.compile()
    return nc


def _get_program(repeat=1):
    key = (repeat, VARIANT)
    if key not in _PROGRAM:
        _PROGRAM[key] = _build_program(repeat)
    return _PROGRAM[key]


def _prep_shared(edge_indices, edge_masks, W_edge, b_edge, ln_scale, ln_bias,
                 W1, b1, W2, b2):
    """Host-side layout prep shared by all cores (weights, constants)."""
    W_pe = np.empty((128, E * 4, 128), np.float32)
    for i in range(E):
        for c in range(2):
            for m in range(2):
                W_pe[:, (i * 2 + c) * 2 + m, :] = W_edge[
                    i, c * 128 : (c + 1) * 128, m * 128 : (m + 1) * 128
                ]
    W1b = np.empty((128, 8, 128), np.float32)
    for k in range(4):
        for m in range(2):
            W1b[:, k * 2 + m, :] = W1[k * 128 : (k + 1) * 128, m * 128 : (m + 1) * 128]
    W2b = np.empty((128, 4, 128), np.float32)
    for k in range(2):
        for m in range(2):
            W2b[:, k * 2 + m, :] = W2[k * 128 : (k + 1) * 128, m * 128 : (m + 1) * 128]

    shared = dict(
        w_pe=W_pe.astype(BF16),
        w1=W1b.astype(BF16),
        w2=W2b.astype(BF16),
        ident=np.eye(128, dtype=BF16),
        ones1=np.ones((128, 1), BF16),
        b1pc=np.ascontiguousarray(b1.reshape(2, 128).T.astype(np.float32)),
        b2pc=np.ascontiguousarray(b2.reshape(2, 128).T.astype(np.float32)),
        lnspc=np.ascontiguousarray(ln_scale.reshape(4, 128).T.astype(np.float32)),
        lnbpc=np.ascontiguousarray(ln_bias.reshape(4, 128).T.astype(np.float32)),
        bedgpc=np.ascontiguousarray(
            b_edge.T.reshape(2, 128, E).transpose(1, 0, 2).astype(np.float32)
        ),
    )

    # per node-half: gather indices (mask-select -> zero row), node-major masks
    halves = []
    for half in range(2):
        n0 = half * NLOC
        idx = edge_indices[:, n0 : n0 + NLOC, :].astype(np.int64)  # [E, NLOC, DEG]
        msk = edge_masks[:, n0 : n0 + NLOC, :]
        idx = np.where(idx < 0, 0, idx)
        sel = np.where(msk > 0, idx, ZROW).astype(np.int32)
        sel = np.concatenate(
            [sel, np.full((E, NPAD - NLOC, DEG), ZROW, np.int32)], axis=1
        )  # [E, NPAD, DEG]
        # per (edge type, 128-node block): j = d*128 + n, wrapped, replicated
        selT = sel.transpose(0, 2, 1)  # [E, DEG, NPAD]
        blocks = selT.reshape(E, DEG, NB1, 128).transpose(0, 2, 1, 3)
        L = blocks.reshape(E, NB1, GIDX)  # j = d*128 + n
        Wv = L.reshape(E, NB1, GIDX // 16, 16).transpose(0, 1, 3, 2)
        idxw = np.tile(Wv, (1, 1, 8, 1)).astype(np.int16)  # [E, NB1, 128, 128]

        mpad = np.concatenate(
            [msk, np.zeros((E, NPAD - NLOC, DEG), np.float32)], axis=1
        )  # [E, NPAD, DEG]
        masknm = np.ascontiguousarray(
            mpad.reshape(E, NB1, 128, DEG).transpose(2, 0, 1, 3).astype(np.float32)
        )  # [128, E, NB1, DEG]
        halves.append((idxw, masknm))
    return shared, halves


def _prep_core(h, shared, halves, b, half):
    n0 = half * NLOC
    tbl = np.concatenate(
        [h[b].astype(BF16), np.zeros((1, F), BF16)], axis=0
    )  # [N+1, F]
    hTl = np.zeros((F, NPAD), np.float32)
    hTl[:, :NLOC] = h[b].T[:, n0 : n0 + NLOC]
    idxw, masknm = halves[half]
    m = dict(tbl=tbl, hT=hTl, idxw=idxw, masknm=masknm)
    m.update(shared)
    return m


def kernel(**inputs):
    h = np.asarray(inputs["h"], np.float32)
    nc = _get_program()
    shared, halves = _prep_shared(
        np.asarray(inputs["edge_indices"]),
        np.asarray(inputs["edge_masks"], np.float32),
        np.asarray(inputs["W_edge"], np.float32),
        np.asarray(inputs["b_edge"], np.float32),
        np.asarray(inputs["ln_scale"], np.float32),
        np.asarray(inputs["ln_bias"], np.float32),
        np.asarray(inputs["W1"], np.float32),
        np.asarray(inputs["b1"], np.float32),
        np.asarray(inputs["W2"], np.float32),
        np.asarray(inputs["b2"], np.float32),
    )
    in_maps = []
    for core in range(8):
        b, half = divmod(core, 2)
        in_maps.append(_prep_core(h, shared, halves, b, half))

    res = run_bass_kernel_spmd(nc, in_maps, core_ids=list(range(8)))

    out = np.empty((B, N, F), np.float32)
    for core in range(8):
        b, half = divmod(core, 2)
        n0 = half * NLOC
        out[b, n0 : n0 + NLOC, :] = res.results[core]["outT"][:, :NLOC].T
    return out

